# revision 10
# baseline (speedup 1.0000x reference)
"""Trainium2 Bass kernel for nn_ConvexOptimExtractor (FISTA sparse coding + dict update).

Strategy: shard the dictionary (M=3072) across 8 cores (384 atoms each), keep the
full batch B=128 on every core so every matmul has a 128-wide moving operand.
Each FISTA iteration does:
  MM1: partial_recons = basis_loc @ ahat_y_loc            (per-core, [768,128])
  AllReduce(partial_recons) over the 8 cores
  Res = I - recons
  MM2: eta*grad_loc = (eta*basis_loc).T @ Res             (per-core, [384,128])
  soft-threshold + Nesterov momentum (elementwise, per-core)
Layouts are chosen so no transposes are needed inside the loop:
  - M-major [128, 3*128] for ahat/ahat_y (partition = atom%128)
  - D-major [128, 6*128] for I/recons/Res (partition = d%128)
  - w1 = basis_loc.T packed M-major (MM1 stationary operand)
  - w2 = eta*basis_loc packed D-major (MM2 stationary operand)
The epilogue (dict update, energies, activity stats) runs once on-device.

kernel(**inputs) takes FULL inputs and returns the FULL output tuple, matching
reference.reference().
"""

import os
import sys

import numpy as np

for _p in ("/opt/trn_rl_repo", "/root/.axon_site/_ro/trn_rl_repo"):
    if os.path.isdir(_p) and _p not in sys.path:
        sys.path.append(_p)

import concourse.bass as bass  # noqa: E402
import concourse.bacc as bacc  # noqa: E402
import concourse.mybir as mybir  # noqa: E402
import concourse.tile as tile  # noqa: E402
from concourse.bass_utils import run_bass_kernel_spmd  # noqa: E402

F32 = mybir.dt.float32

D, M, B = 768, 3072, 128
NC = 8
ML = M // NC          # 384 atoms per core
KM = ML // 128        # 3 M-chunks per core
KD = D // 128         # 6 D-chunks
REG = 0.1
NUM_ITER = int(os.environ.get("FISTA_NUM_ITER", "500"))
AHL = 300.0
LOWEST_ACT = 0.001
STEP_SIZE = 0.005
DECAY = np.float32((AHL - 1.0) / AHL)


def _momentum_coeffs(num_iter):
    cs, tk = [], 1.0
    for _ in range(num_iter):
        tk_n = (1.0 + np.sqrt(1.0 + 4.0 * tk**2)) / 2.0
        cs.append((tk - 1.0) / tk_n)
        tk = tk_n
    return np.array(cs, dtype=np.float32)


# ------------------------- layout helpers (host side) -------------------------

def to_mmaj(x):
    """[ML, N] -> [128, KM*N]  (partition = row%128, col = (row//128)*N + c)."""
    r, n = x.shape
    k = r // 128
    return np.ascontiguousarray(x.reshape(k, 128, n).transpose(1, 0, 2).reshape(128, k * n))


def from_mmaj(x, n):
    k = x.shape[1] // n
    return np.ascontiguousarray(x.reshape(128, k, n).transpose(1, 0, 2).reshape(k * 128, n))


# ------------------------------ kernel builder --------------------------------

def build_kernel(num_iter, eta, coeffs):
    nc = bacc.Bacc(None, target_bir_lowering=False)

    def din(name, shape):
        return nc.dram_tensor(name, shape, F32, kind="ExternalInput")

    def dout(name, shape):
        return nc.dram_tensor(name, shape, F32, kind="ExternalOutput")

    w1_d = din("w1", [128, KM * D])        # basis_loc.T, M-major
    w2_d = din("w2", [128, KD * ML])       # eta*basis_loc, D-major
    isb_d = din("isb", [128, D])           # I = batch.T, D-major
    phi_d = din("phi", [128, KD * ML])     # PHI_loc, D-major
    idn_d = din("idn", [128, 128])         # identity (PE transpose)
    invf_d = din("invf", [128, 1])         # 1/batch_freq
    al_d = din("al_in", [128, KM])         # ActL1_loc, M-major [p, k]
    hs_d = din("hs_in", [128, KM])         # HessianDiag_loc, M-major

    dbg = os.environ.get("FISTA_DEBUG") == "1"
    if dbg:
        rt_o = dout("rt_o", [128, D])
        at_o = dout("at_o", [128, ML])
        dbr_o = dout("dbr_o", [128, KD * ML])
        hbc_o = dout("hbc_o", [128, ML])
        nrm_o = dout("nrm_o", [128, KM])
    ahat_o = dout("ahat_o", [128, ML])
    rec_o = dout("rec_o", [128, D])
    nb_o = dout("nb_o", [128, KD * ML])
    hs_o = dout("hs_o", [128, KM])
    al_o = dout("al_o", [128, KM])
    en_o = dout("en_o", [1, 2])

    ethr = float(np.float32(eta) * np.float32(REG))
    groups = [list(range(NC))]

    with tile.TileContext(nc) as tc:
        with (
            tc.tile_pool(name="sb", bufs=1) as sb,
            tc.tile_pool(name="tmp", bufs=2) as tp,
            tc.tile_pool(name="ps", bufs=2, space="PSUM") as ps,
            tc.tile_pool(name="dram", bufs=2, space="DRAM") as dr,
        ):
            cst = sb.tile([128, 2], F32, tag="cst")
            nc.vector.memset(cst[:, 0:1], 0.0)
            nc.vector.memset(cst[:, 1:2], -ethr)
            nc.const_aps.aps[(F32, 0.0)] = cst[:, 0:1]
            nc.const_aps.aps[(F32, float(-ethr))] = cst[:, 1:2]

            w1 = sb.tile([128, KM * D], F32, tag="w1")
            w2 = sb.tile([128, KD * ML], F32, tag="w2")
            isb = sb.tile([128, D], F32, tag="isb")
            y = sb.tile([128, ML], F32, tag="y")
            a0 = sb.tile([128, ML], F32, tag="a0")
            a1 = sb.tile([128, ML], F32, tag="a1")
            S = sb.tile([128, D], F32, tag="S")      # allreduced recons
            R = sb.tile([128, D], F32, tag="R")      # residual

            nc.sync.dma_start(w1[:], w1_d[:])
            nc.sync.dma_start(w2[:], w2_d[:])
            nc.sync.dma_start(isb[:], isb_d[:])
            nc.vector.memset(y[:], 0.0)
            nc.vector.memset(a0[:], 0.0)

            a_prev, a_cur = a0, a1

            def recons_allreduce(rhs, tag):
                """basis_loc @ rhs -> allreduce -> S."""
                p1 = ps.tile([128, D], F32, tag="p1")
                for d in range(KD):
                    for k in range(KM):
                        nc.tensor.matmul(
                            p1[:, d * 128:(d + 1) * 128],
                            w1[:, k * D + d * 128: k * D + (d + 1) * 128],
                            rhs[:, k * 128:(k + 1) * 128],
                            start=(k == 0),
                            stop=(k == KM - 1),
                        )
                part = tp.tile([128, D], F32, tag="part")
                nc.vector.tensor_copy(part[:], p1[:])
                bi = dr.tile([128, D], F32, tag="bi" + tag)
                bo = dr.tile([128, D], F32, tag="bo" + tag)
                nc.sync.dma_start(bi[:], part[:])
                nc.gpsimd.collective_compute(
                    "AllReduce",
                    mybir.AluOpType.add,
                    replica_groups=groups,
                    ins=[bi.opt()],
                    outs=[bo.opt()],
                )
                nc.sync.dma_start(S[:], bo[:])

            for t in range(num_iter):
                recons_allreduce(y, "l")
                nc.vector.tensor_sub(R[:], isb[:], S[:])

                p2 = ps.tile([128, ML], F32, tag="p2")
                for m in range(KM):
                    for d in range(KD):
                        nc.tensor.matmul(
                            p2[:, m * 128:(m + 1) * 128],
                            w2[:, d * ML + m * 128: d * ML + (m + 1) * 128],
                            R[:, d * 128:(d + 1) * 128],
                            start=(d == 0),
                            stop=(d == KD - 1),
                        )
                yg = tp.tile([128, ML], F32, tag="yg")
                nc.vector.tensor_add(yg[:], y[:], p2[:])          # y + eta*grad
                nc.scalar.activation(a_cur[:], yg[:],
                                     mybir.ActivationFunctionType.Relu,
                                     bias=-ethr)                   # soft-threshold
                c = float(coeffs[t])
                dd = tp.tile([128, ML], F32, tag="dd")
                nc.vector.tensor_sub(dd[:], a_cur[:], a_prev[:])
                dd2 = tp.tile([128, ML], F32, tag="dd2")
                nc.scalar.mul(dd2[:], dd[:], c)
                nc.vector.tensor_add(y[:], a_cur[:], dd2[:])
                a_prev, a_cur = a_cur, a_prev

            ah = a_prev  # final ahat (after last swap, a_prev == last written)

            # ---------------- epilogue ----------------
            recons_allreduce(ah, "e")
            nc.vector.tensor_sub(R[:], isb[:], S[:])
            nc.sync.dma_start(rec_o[:], S[:])
            nc.sync.dma_start(ahat_o[:], ah[:])

            idn = sb.tile([128, 128], F32, tag="idn")
            invf = sb.tile([128, 1], F32, tag="invf")
            ones = sb.tile([128, 1], F32, tag="ones")
            al_i = sb.tile([128, KM], F32, tag="al_i")
            hs_i = sb.tile([128, KM], F32, tag="hs_i")
            nc.sync.dma_start(idn[:], idn_d[:])
            nc.sync.dma_start(invf[:], invf_d[:])
            nc.sync.dma_start(al_i[:], al_d[:])
            nc.sync.dma_start(hs_i[:], hs_d[:])
            nc.vector.memset(ones[:], 1.0)

            # energies: sum(I^2), sum(Res^2)
            sqt = tp.tile([128, D], F32, tag="sqt")
            acc = sb.tile([128, 2], F32, tag="acc")
            nc.scalar.activation(sqt[:], isb[:], mybir.ActivationFunctionType.Square,
                                 accum_out=acc[:, 0:1])
            nc.scalar.activation(sqt[:], R[:], mybir.ActivationFunctionType.Square,
                                 accum_out=acc[:, 1:2])
            en = sb.tile([1, 2], F32, tag="en")
            nc.gpsimd.tensor_reduce(en[0:1, 0:1], acc[:, 0:1],
                                    axis=mybir.AxisListType.C, op=mybir.AluOpType.add)
            nc.gpsimd.tensor_reduce(en[0:1, 1:2], acc[:, 1:2],
                                    axis=mybir.AxisListType.C, op=mybir.AluOpType.add)
            nc.sync.dma_start(en_o[:], en[:])

            # ActL1_new / Hess_new  (ahat >= 0 so |ahat| = ahat)
            asum = sb.tile([128, KM], F32, tag="asum")
            hsum = sb.tile([128, KM], F32, tag="hsum")
            sq2 = tp.tile([128, 128], F32, tag="sq2")
            for k in range(KM):
                nc.vector.reduce_sum(asum[:, k:k + 1], ah[:, k * 128:(k + 1) * 128],
                                     axis=mybir.AxisListType.X)
                nc.scalar.activation(sq2[:], ah[:, k * 128:(k + 1) * 128],
                                     mybir.ActivationFunctionType.Square,
                                     accum_out=hsum[:, k:k + 1])
            al_n = sb.tile([128, KM], F32, tag="al_n")
            hs_n = sb.tile([128, KM], F32, tag="hs_n")
            t1 = tp.tile([128, KM], F32, tag="t1")
            nc.scalar.mul(t1[:], al_i[:], float(DECAY))
            nc.vector.tensor_scalar(al_n[:], asum[:], 1.0 / (B * AHL), None,
                                    op0=mybir.AluOpType.mult)
            nc.vector.tensor_add(al_n[:], al_n[:], t1[:])
            nc.scalar.mul(t1[:], hs_i[:], float(DECAY))
            nc.vector.tensor_scalar(hs_n[:], hsum[:], 1.0 / (B * AHL), None,
                                    op0=mybir.AluOpType.mult)
            nc.vector.tensor_add(hs_n[:], hs_n[:], t1[:])
            nc.sync.dma_start(al_o[:], al_n[:])
            nc.sync.dma_start(hs_o[:], hs_n[:])

            # Res.T (B-major), scaled by 1/freq
            rt = sb.tile([128, D], F32, tag="rt")
            for d in range(KD):
                pt = ps.tile([128, 128], F32, tag="p2")
                nc.tensor.transpose(pt[:], R[:, d * 128:(d + 1) * 128], idn[:])
                nc.vector.tensor_copy(rt[:, d * 128:(d + 1) * 128], pt[:])
            nc.vector.tensor_scalar(rt[:], rt[:], invf[:, 0:1], None,
                                    op0=mybir.AluOpType.mult)
            # ahat.T (B-major)
            at = sb.tile([128, ML], F32, tag="at")
            for k in range(KM):
                pt = ps.tile([128, 128], F32, tag="p2")
                nc.tensor.transpose(pt[:], ah[:, k * 128:(k + 1) * 128], idn[:])
                nc.vector.tensor_copy(at[:, k * 128:(k + 1) * 128], pt[:])

            # dBasis (D-major) = STEP/B * Res_scaled @ ahat.T
            db = sb.tile([128, KD * ML], F32, tag="db")
            for d in range(KD):
                pd = ps.tile([128, ML], F32, tag="p2")
                nc.tensor.matmul(pd[:], rt[:, d * 128:(d + 1) * 128], at[:],
                                 start=True, stop=True)
                nc.scalar.mul(db[:, d * ML:(d + 1) * ML], pd[:], STEP_SIZE / B)

            if dbg:
                nc.sync.dma_start(rt_o[:], rt[:])
                nc.sync.dma_start(at_o[:], at[:])
                nc.sync.dma_start(dbr_o[:], db[:])

            # 1/(Hess_new + LOWEST_ACT), broadcast to [128, ML] in column-major order
            hd = sb.tile([128, KM], F32, tag="hd")
            nc.vector.tensor_scalar(hd[:], hs_n[:], LOWEST_ACT, None,
                                    op0=mybir.AluOpType.add)
            hb_d = dr.tile([128, KM], F32, tag="hb")
            nc.sync.dma_start(hb_d[:], hd[:])
            hrow = sb.tile([1, ML], F32, tag="hrow")
            for k in range(KM):
                nc.sync.dma_start(hrow[0:1, k * 128:(k + 1) * 128], hb_d[:, k:k + 1])
            nc.vector.reciprocal(hrow[:], hrow[:])
            hbc = sb.tile([128, ML], F32, tag="hbc")
            nc.gpsimd.partition_broadcast(hbc[:], hrow[0:1, :])

            if dbg:
                nc.sync.dma_start(hbc_o[:], hbc[:])

            # new_basis = normalize_cols(PHI + dBasis/(hess+eps))
            phi = sb.tile([128, KD * ML], F32, tag="phi")
            nc.sync.dma_start(phi[:], phi_d[:])
            nb = sb.tile([128, KD * ML], F32, tag="nb")
            for d in range(KD):
                sl = slice(d * ML, (d + 1) * ML)
                nc.vector.tensor_mul(db[:, sl], db[:, sl], hbc[:])
                nc.vector.tensor_add(nb[:, sl], phi[:, sl], db[:, sl])
            sq3 = tp.tile([128, KD * ML], F32, tag="sq3")
            nc.scalar.activation(sq3[:], nb[:], mybir.ActivationFunctionType.Square)
            pn = ps.tile([128, KM], F32, tag="p2")
            for k in range(KM):
                for d in range(KD):
                    nc.tensor.matmul(
                        pn[:, k:k + 1],
                        sq3[:, d * ML + k * 128: d * ML + (k + 1) * 128],
                        ones[:],
                        start=(d == 0),
                        stop=(d == KD - 1),
                    )
            nrm = sb.tile([128, KM], F32, tag="nrm")
            nc.scalar.activation(nrm[:], pn[:], mybir.ActivationFunctionType.Sqrt)
            if dbg:
                nc.sync.dma_start(nrm_o[:], nrm[:])
            nb_bd = dr.tile([128, KM], F32, tag="nbb")
            nc.sync.dma_start(nb_bd[:], nrm[:])
            nrow = sb.tile([1, ML], F32, tag="nrow")
            for k in range(KM):
                nc.sync.dma_start(nrow[0:1, k * 128:(k + 1) * 128], nb_bd[:, k:k + 1])
            nc.vector.reciprocal(nrow[:], nrow[:])
            nbc = sb.tile([128, ML], F32, tag="nbc")
            nc.gpsimd.partition_broadcast(nbc[:], nrow[0:1, :])
            for d in range(KD):
                sl = slice(d * ML, (d + 1) * ML)
                nc.vector.tensor_mul(nb[:, sl], nb[:, sl], nbc[:])
            nc.sync.dma_start(nb_o[:], nb[:])

    nc.compile()
    return nc


_CACHE = {}


def _get_kernel(num_iter, eta, coeffs):
    key = (num_iter, float(eta))
    if key not in _CACHE:
        _CACHE[key] = build_kernel(num_iter, eta, coeffs)
    return _CACHE[key]


def kernel(batch, batch_freq, PHI, HessianDiag, ActL1, num_iter=NUM_ITER):
    batch = np.asarray(batch, np.float32)
    batch_freq = np.asarray(batch_freq, np.float32)
    PHI = np.asarray(PHI, np.float32)
    HessianDiag = np.asarray(HessianDiag, np.float32)
    ActL1 = np.asarray(ActL1, np.float32)

    I = np.ascontiguousarray(batch.T)                      # [D, B]
    G = (PHI @ PHI.T).astype(np.float32)
    L = float(np.linalg.eigvalsh(G.astype(np.float64)).max())
    eta = float(np.float32(1.0) / np.float32(L))
    coeffs = _momentum_coeffs(num_iter)

    nc = _get_kernel(num_iter, eta, coeffs)

    isb = to_mmaj(I)                                       # D-major [128, 768]
    idn = np.eye(128, dtype=np.float32)
    invf = (1.0 / batch_freq).astype(np.float32).reshape(128, 1)

    in_maps = []
    for c in range(NC):
        bl = PHI[:, c * ML:(c + 1) * ML]                   # [768, 384]
        w1 = to_mmaj(np.ascontiguousarray(bl.T))        # [128, 3*768]
        w2 = to_mmaj(np.ascontiguousarray(np.float32(eta) * bl))  # [128, 6*384]
        phi_l = to_mmaj(bl)
        al = ActL1[c * ML:(c + 1) * ML].reshape(KM, 128).T
        hs = HessianDiag[c * ML:(c + 1) * ML].reshape(KM, 128).T
        in_maps.append({
            "w1": np.ascontiguousarray(w1),
            "w2": np.ascontiguousarray(w2),
            "isb": isb,
            "phi": np.ascontiguousarray(phi_l),
            "idn": idn,
            "invf": invf,
            "al_in": np.ascontiguousarray(al),
            "hs_in": np.ascontiguousarray(hs),
        })

    res = run_bass_kernel_spmd(nc, in_maps, core_ids=list(range(NC)))
    outs = res.results

    recons = from_mmaj(outs[0]["rec_o"], B)                # [768, 128]
    ahat = np.concatenate([from_mmaj(outs[c]["ahat_o"], B) for c in range(NC)], axis=0)
    new_basis = np.concatenate(
        [from_mmaj(outs[c]["nb_o"], ML) for c in range(NC)], axis=1)
    hess = np.concatenate([outs[c]["hs_o"].T.reshape(ML) for c in range(NC)])
    actl1 = np.concatenate([outs[c]["al_o"].T.reshape(ML) for c in range(NC)])
    sE, nE = outs[0]["en_o"][0]
    snr = np.float32(np.float32(sE / AHL) / np.float32(nE / AHL))

    return (recons, ahat, snr,
            np.float32(actl1.max()), np.float32(actl1.min()),
            new_basis, hess)
